# revision 47
# baseline (speedup 1.0000x reference)
"""MultiHeadTimeDimensionAttention kernel for Trainium2 (8 NeuronCores).

Math (per batch b):
  q[h,d]      = o_last[b] . Wq[h,:,d] + bq[h,d]          (host, 0.4% of FLOPs)
  wkq[z,h]    = sum_d Wk[h,z,d] q[h,d]                   (host)
  scores[t,h] = sum_z o_all[b,t,z] * wkq[z,h]            (device, bk drops: softmax-invariant)
  p = exp(scores - max_t), L = sum_t p
  r[h,z]      = sum_t p[t,h] * o_all[b,t,z]
  ctx[h,d]    = (sum_z r[h,z] Wv[h,z,d]) / L[h] + bv[h,d]

Device layout: scores^T kept in a (tb,h)-packed [128, 512] PSUM layout via
column-tiled (tile_position) M=16 matmuls, 4 t-blocks concurrent on the PE
array.  Softmax runs at full 128-partition parallelism; cross-partition
head reductions go through tiny PE transposes + a K=1 broadcast matmul.
A^T tiles are produced on-PE from the (single) natural-layout copy of
o_all; fp16 everywhere on the PE, fp32 PSUM/softmax.

The two batches per core are software-pipelined: batch 1's transpose fills
and score matmuls are emitted between batch 0's stages so the PE stays busy
during batch 0's softmax and the DMA stream stays ahead of compute.

Sharding: data-parallel over B; each core handles B/8 = 2 batches.
"""

import numpy as np

import concourse.bacc as bacc
import concourse.tile as tile
import concourse.mybir as mybir
from concourse.bass_utils import run_bass_kernel_spmd
from concourse.masks import make_identity

B, T, Z, H, DK = 16, 4096, 1024, 16, 64
P = 128
NCORES = 8
BLOC = B // NCORES          # batches per core
ZC = Z // P                 # 8 z-chunks
TB = 512                    # t-block (one PSUM bank column span)
NTBG = 2                    # two groups of 4 t-blocks per batch
F32 = mybir.dt.float32
F16 = mybir.dt.float16
EXP = mybir.ActivationFunctionType.Exp
AX = mybir.AxisListType.X
MULT = mybir.AluOpType.mult


def build_nc():
    nc = bacc.Bacc(None, target_bir_lowering=False)

    o16 = nc.declare_dram_parameter("o16", [BLOC, T, Z], F16, isOutput=False)
    oT16 = nc.declare_dram_parameter("oT16", [2, P, ZC, TB], F16, isOutput=False)
    wv16 = nc.declare_dram_parameter("Wv16", [P, ZC, Z], F16, isOutput=False)
    wkq16 = nc.declare_dram_parameter("wkq16", [BLOC, P, ZC, H], F16, isOutput=False)
    bv128 = nc.declare_dram_parameter("bv128", [P, DK], F32, isOutput=False)
    dmask = nc.declare_dram_parameter("dmask", [P, 256], F16, isOutput=False)
    out = nc.declare_dram_parameter("out", [BLOC, Z], F32, isOutput=True)

    with tile.TileContext(nc) as tc:
        with (
            tc.tile_pool(name="const", bufs=1) as const,
            tc.tile_pool(name="abuf", bufs=2) as abuf,
            tc.tile_pool(name="atbuf", bufs=1) as atbuf,
            tc.tile_pool(name="stage", bufs=2) as stage,
            tc.tile_pool(name="small", bufs=2) as small,
            tc.tile_pool(name="scp", bufs=2, space="PSUM") as scp,
            tc.tile_pool(name="atp", bufs=3, space="PSUM") as atp,
            tc.tile_pool(name="xps", bufs=1, space="PSUM") as xps,
            tc.tile_pool(name="rp", bufs=1, space="PSUM") as rp,
        ):
            ident16 = const.tile([P, P], F16)
            make_identity(nc, ident16)
            identf = const.tile([P, P], F32)
            make_identity(nc, identf)
            onesf = const.tile([1, 1], F32)
            nc.vector.memset(onesf, 1.0)
            negones = const.tile([1, 1], F32)
            nc.vector.memset(negones, -1.0)

            # ------------- DMA schedule (single sync ring, FIFO starts) -----
            # tiny first, then b0 stream, then b1 stream with wv interleaved
            wkq_sb = []
            for b in range(BLOC):
                wkq_b = const.tile([P, ZC, H], F16, tag=f"wkq{b}")
                nc.sync.dma_start(out=wkq_b, in_=wkq16[b])
                wkq_sb.append(wkq_b)
            dmask_sb = const.tile([P, 256], F16)
            nc.sync.dma_start(out=dmask_sb, in_=dmask[:])
            bv_sb = const.tile([P, DK], F32)
            nc.sync.dma_start(out=bv_sb, in_=bv128[:])

            wv_sb = const.tile([P, ZC, Z], F16)
            a_sbs = []

            def load_blocks(b, blks, split=1):
                for blk in blks:
                    for s in range(split):
                        g0 = blk * 4 + s * 4 // split
                        g1 = blk * 4 + (s + 1) * 4 // split
                        nc.sync.dma_start(
                            out=a_sbs[b][:, g0:g1, :],
                            in_=o16[b, g0 * P : g1 * P, :].rearrange(
                                "(i zp) z -> zp i z", zp=P
                            ),
                        )

            for b in range(BLOC):
                a_sb = abuf.tile([P, 32, Z], F16, tag="a", name=f"a_sb{b}")
                a_sbs.append(a_sb)
            load_blocks(0, range(2), split=4)
            load_blocks(0, range(2, 8))
            load_blocks(1, range(0, 6))
            # host-pre-transposed A^T strips for b1/tbg1 j=2,3: replace 128 PE
            # transposes on the tail path; must land before S11's matmuls
            atx_sb = const.tile([P, 2, ZC, TB], F16)
            for jj in range(2):
                nc.sync.dma_start(out=atx_sb[:, jj], in_=oT16[jj])
            load_blocks(1, range(6, 8))
            # wv is consumed last (ctx matmuls run after both r passes)
            for zc in range(ZC):
                nc.sync.dma_start(out=wv_sb[:, zc, :], in_=wv16[:, zc, :])

            # ------------- stage emitters ----------------------------------
            def emit_tbg(b, tbg, mid=None, xbar_js=None):
                """A^T fills via PE transposes (j-outer), then col-tiled score
                matmuls; strips listed in xbar_js come pre-transposed in atx."""
                xbar_js = xbar_js or {}
                at_sb = atbuf.tile([P, 4, ZC, TB], F16, tag="at")
                sc_ps = scp.tile([P, TB], F32, tag="sc")
                nfill = 0
                for j in range(4):
                    if j in xbar_js:
                        continue
                    for zcp in range(4):
                        at_ps = atp.tile([P, 8, P], F16, tag="atp")
                        for zz in range(2):
                            zc = 2 * zcp + zz
                            for i in range(4):
                                gi = (tbg * 4 + j) * 4 + i
                                nc.tensor.transpose(
                                    at_ps[:, 4 * zz + i, :],
                                    a_sbs[b][:, gi, zc * P : (zc + 1) * P],
                                    ident16,
                                )
                        eng = nc.scalar if nfill % 3 == 2 else nc.vector
                        cp = eng.copy if nfill % 3 == 2 else eng.tensor_copy
                        cp(
                            out=at_sb[:, j, 2 * zcp : 2 * zcp + 2, :],
                            in_=at_ps.rearrange("p (zz i) c -> p zz (i c)", zz=2),
                        )
                        nfill += 1
                    if j == 1 and mid is not None:
                        mid()
                for zc in range(ZC):
                    for j in range(4):
                        rhs = (
                            atx_sb[:, xbar_js[j], zc, :]
                            if j in xbar_js
                            else at_sb[:, j, zc, :]
                        )
                        nc.tensor.matmul(
                            sc_ps[32 * j : 32 * j + 16, :],
                            wkq_sb[b][:, zc, :],
                            rhs,
                            start=(zc == 0),
                            stop=(zc == ZC - 1),
                            tile_position=(0, 32 * j),
                        )
                return sc_ps

            def emit_sm_maxes(b, sc_tiles):
                """per-partition chunk maxes (vector engine only)."""
                m_sb = small.tile([P, 2], F32, tag="m")
                for tbg in range(NTBG):
                    nc.vector.reduce_max(
                        m_sb[:, tbg : tbg + 1], sc_tiles[tbg], axis=AX
                    )
                mm1 = small.tile([P, 1], F32, tag="mm1")
                nc.vector.reduce_max(mm1, m_sb, axis=AX)
                return mm1

            def emit_sm_exp(b, sc_tiles, mm1):
                """per-head max combine + exp; scores are [32*(tb%4)+h, 512]."""
                p_sb = stage.tile([P, NTBG, TB], F16, tag="p")
                xs = xps.tile([P, P], F32, tag="xs")
                nc.tensor.transpose(xs[0:1, :], mm1, identf)
                M32 = small.tile([1, 32], F32, tag="M32")
                nc.vector.reduce_max(
                    M32, xs[0:1, :].rearrange("a (j c) -> a c j", j=4), axis=AX
                )
                Mr = small.tile([1, 4, 32], F32, tag="Mr")
                nc.vector.tensor_copy(Mr, M32.unsqueeze(1).to_broadcast((1, 4, 32)))
                xs = xps.tile([P, P], F32, tag="xs")
                nc.tensor.matmul(xs[:, 0:1], Mr, negones, start=True, stop=True)
                negM128 = small.tile([P, 1], F32, tag="negM128")
                nc.vector.tensor_copy(negM128, xs[:, 0:1])
                ls_sb = small.tile([P, 2], F32, tag="ls")
                for tbg in range(NTBG):
                    nc.scalar.activation(
                        out=p_sb[:, tbg, :],
                        in_=sc_tiles[tbg],
                        func=EXP,
                        bias=negM128,
                        scale=1.0,
                        accum_out=ls_sb[:, tbg : tbg + 1],
                    )
                return p_sb, ls_sb

            def emit_sm_l(b, ls_sb):
                """L = per-head sum of chunk sums; rinv128 = 1/L per partition."""
                ls1 = small.tile([P, 1], F32, tag="ls1")
                nc.vector.reduce_sum(ls1, ls_sb, axis=AX)
                xs = xps.tile([P, P], F32, tag="xs")
                nc.tensor.transpose(xs[0:1, :], ls1, identf)
                L32 = small.tile([1, 32], F32, tag="L32")
                nc.vector.reduce_sum(
                    L32, xs[0:1, :].rearrange("a (j c) -> a c j", j=4), axis=AX
                )
                rinv32 = small.tile([1, 32], F32, tag="rinv32")
                nc.vector.reciprocal(rinv32, L32)
                rinvr = small.tile([1, 4, 32], F32, tag="rinvr")
                nc.vector.tensor_copy(
                    rinvr, rinv32.unsqueeze(1).to_broadcast((1, 4, 32))
                )
                xs = xps.tile([P, P], F32, tag="xs")
                nc.tensor.matmul(xs[:, 0:1], rinvr, onesf, start=True, stop=True)
                rinv128 = small.tile([P, 1], F32, tag="rinv128")
                nc.vector.tensor_copy(rinv128, xs[:, 0:1])
                return rinv128

            def emit_pt(b, p_sb):
                """p natural (t on partitions) via PE transposes."""
                ptT = []
                for tbg in range(NTBG):
                    pt_ps = xps.tile([P, 4, P], F16, tag="ptT")
                    for i in range(4):
                        nc.tensor.transpose(
                            pt_ps[:, i, :],
                            p_sb[:, tbg, i * P : (i + 1) * P],
                            ident16,
                        )
                    pt_sb = stage.tile([P, 4, P], F16, tag=f"ptT{tbg}", bufs=1)
                    nc.vector.tensor_copy(pt_sb, pt_ps)
                    ptT.append(pt_sb)
                return ptT

            def emit_r(b, ptT):
                """r[h, z] col-tiled over z-quarters; rt = r^T chunks."""
                r_ps = rp.tile([P, 256], F32, tag="r")
                nmm = 0
                for tbg in range(NTBG):
                    for i in range(4):
                        for jt in range(4):
                            gi = (tbg * 4 + jt) * 4 + i
                            for j in range(4):
                                nc.tensor.matmul(
                                    r_ps[32 * j : 32 * j + 16, :],
                                    ptT[tbg][:, i, 32 * jt : 32 * jt + 16],
                                    a_sbs[b][:, gi, j * 256 : (j + 1) * 256],
                                    start=(nmm == 0),
                                    stop=(nmm == 31),
                                    tile_position=(0, 32 * j),
                                )
                            nmm += 1
                r16 = stage.tile([P, 256], F16, tag="r16", bufs=1)
                nc.scalar.copy(out=r16, in_=r_ps)
                rT_ps = xps.tile([P, 4, P], F16, tag="ptT")
                for half in range(2):
                    nc.tensor.transpose(
                        rT_ps[:, half, :],
                        r16[:, half * P : (half + 1) * P],
                        ident16,
                    )
                rt_sb = stage.tile([P, 2, P], F16, tag="rt")
                nc.scalar.copy(out=rt_sb, in_=rT_ps[:, 0:2, :])
                return rt_sb

            def emit_ctx(b, rt_sb, rinv128):
                cf_ps = rp.tile([P, 256], F32, tag="r", name="cf_ps")
                for zc in range(ZC):
                    half, zq = zc % 2, zc // 2
                    for j in range(4):
                        nc.tensor.matmul(
                            cf_ps[32 * j : 32 * j + 16, :],
                            rt_sb[:, half, 32 * zq : 32 * zq + 16],
                            wv_sb[:, zc, j * 256 : (j + 1) * 256],
                            start=(zc == 0),
                            stop=(zc == ZC - 1),
                            tile_position=(0, 32 * j),
                        )
                ctxm = stage.tile([P, 256], F32, tag="ctxm", bufs=1)
                nc.vector.scalar_tensor_tensor(
                    out=ctxm, in0=cf_ps, scalar=rinv128, in1=dmask_sb,
                    op0=MULT, op1=MULT,
                )
                ctxs = stage.tile([P, DK], F32, tag="ctxs")
                nc.vector.reduce_sum(
                    ctxs, ctxm.rearrange("p (g d) -> p d g", d=DK), axis=AX
                )
                nc.vector.tensor_add(out=ctxs, in0=ctxs, in1=bv_sb)
                outv = out[b].rearrange("(h d) -> h d", h=H)
                for j in range(4):
                    eng = nc.sync if j % 2 == 0 else nc.scalar
                    eng.dma_start(
                        out=outv[4 * j : 4 * j + 4, :],
                        in_=ctxs[36 * j : 36 * j + 4, :],
                    )

            # ------------- interleaved schedule ----------------------------
            # PE FIFO: T00 T01 [sm0 maxes] T10(sm0-exp after round 0) T11
            #          PT0 R0 [sm1 on V/S] CT0 PT1 R1 CT1
            sc00 = emit_tbg(0, 0)
            sc01 = emit_tbg(0, 1)
            sc0 = [sc00, sc01]
            mm1_0 = emit_sm_maxes(0, sc0)
            hold = {}

            def mid0():
                hold["p0"], hold["ls0"] = emit_sm_exp(0, sc0, mm1_0)
                hold["rinv0"] = emit_sm_l(0, hold["ls0"])

            sc10 = emit_tbg(1, 0, mid=mid0)
            sc11 = emit_tbg(1, 1, xbar_js={2: 0, 3: 1})
            sc1 = [sc10, sc11]
            mm1_1 = emit_sm_maxes(1, sc1)
            ptT0 = emit_pt(0, hold["p0"])
            p1, ls1 = emit_sm_exp(1, sc1, mm1_1)
            rt0 = emit_r(0, ptT0)
            ptT1 = emit_pt(1, p1)
            rt1 = emit_r(1, ptT1)
            rinv1 = emit_sm_l(1, ls1)
            emit_ctx(0, rt0, hold["rinv0"])
            emit_ctx(1, rt1, rinv1)

    nc.finalize()
    return nc


_NC_CACHE = {}


def _get_nc():
    if "nc" not in _NC_CACHE:
        _NC_CACHE["nc"] = build_nc()
    return _NC_CACHE["nc"]


def prep_inputs(o_all, o_last, Wk, Wv, Wq, bk, bv, bq):
    """Host-side shard + layout prep. Returns per-core input maps."""
    o_all = np.asarray(o_all, dtype=np.float32)
    o_last = np.asarray(o_last, dtype=np.float32)
    Wk = np.asarray(Wk, dtype=np.float32)
    Wv = np.asarray(Wv, dtype=np.float32)
    Wq = np.asarray(Wq, dtype=np.float32)
    bv = np.asarray(bv, dtype=np.float32)
    bq = np.asarray(bq, dtype=np.float32)

    # q for all batches, then wkq[z, h] = sum_d Wk[h,z,d] q[h,d]
    wq_flat = Wq.transpose(1, 0, 2).reshape(Z, Z)
    q_all = o_last[:, 0, :] @ wq_flat + bq.reshape(Z)          # [B, Z]
    wkq_all = np.einsum(
        "hzd,bhd->bzh", Wk, q_all.reshape(B, H, DK), optimize=True
    )                                                           # [B, Z, H]

    wv_flat = Wv.transpose(1, 0, 2).reshape(Z, Z)
    wv16 = np.ascontiguousarray(
        wv_flat.reshape(ZC, P, Z).transpose(1, 0, 2)
    ).astype(np.float16)

    bv128 = np.zeros((P, DK), dtype=np.float32)
    dmask = np.zeros((P, 256), dtype=np.float16)
    for h in range(H):
        j, r = h // 4, h % 4
        bv128[36 * j + r] = bv[h]
        dmask[32 * j + h, DK * r : DK * (r + 1)] = 1.0

    in_maps = []
    for c in range(NCORES):
        sl = slice(c * BLOC, (c + 1) * BLOC)
        wkq16 = np.ascontiguousarray(
            wkq_all[sl].reshape(BLOC, ZC, P, H).transpose(0, 2, 1, 3)
        ).astype(np.float16)
        o16c = o_all[sl].astype(np.float16)
        # pre-transposed A^T strips: batch 1, t-blocks 6 and 7
        oT = np.ascontiguousarray(
            np.stack([o16c[1, tb * TB : (tb + 1) * TB, :] for tb in (6, 7)])
            .reshape(2, TB, ZC, P)
            .transpose(0, 3, 2, 1)                    # [2, P, ZC, TB]
        )
        in_maps.append(
            {
                "o16": o16c,
                "oT16": oT,
                "Wv16": wv16,
                "wkq16": wkq16,
                "bv128": bv128,
                "dmask": dmask,
            }
        )
    return in_maps


def kernel(o_all, o_last, Wk, Wv, Wq, bk, bv, bq, _trace=False, _trace_kwargs=None):
    nc = _get_nc()
    in_maps = prep_inputs(o_all, o_last, Wk, Wv, Wq, bk, bv, bq)
    res = run_bass_kernel_spmd(
        nc, in_maps, core_ids=list(range(NCORES)), trace=_trace,
        **(_trace_kwargs or {}),
    )
    outs = [r["out"] for r in res.results]
    full = np.concatenate(outs, axis=0).reshape(B, 1, Z)
    if _trace:
        kernel.last_result = res
    return full


# revision 48
# speedup vs baseline: 1.0339x; 1.0339x over previous
"""MultiHeadTimeDimensionAttention kernel for Trainium2 (8 NeuronCores).

Math (per batch b):
  q[h,d]      = o_last[b] . Wq[h,:,d] + bq[h,d]          (host, 0.4% of FLOPs)
  wkq[z,h]    = sum_d Wk[h,z,d] q[h,d]                   (host)
  scores[t,h] = sum_z o_all[b,t,z] * wkq[z,h]            (device, bk drops: softmax-invariant)
  p = exp(scores - max_t), L = sum_t p
  r[h,z]      = sum_t p[t,h] * o_all[b,t,z]
  ctx[h,d]    = (sum_z r[h,z] Wv[h,z,d]) / L[h] + bv[h,d]

Device layout: scores^T kept in a (tb,h)-packed [128, 512] PSUM layout via
column-tiled (tile_position) M=16 matmuls, 4 t-blocks concurrent on the PE
array.  Softmax runs at full 128-partition parallelism; cross-partition
head reductions go through tiny PE transposes + a K=1 broadcast matmul.
A^T tiles are produced on-PE from the (single) natural-layout copy of
o_all; fp16 everywhere on the PE, fp32 PSUM/softmax.

The two batches per core are software-pipelined: batch 1's transpose fills
and score matmuls are emitted between batch 0's stages so the PE stays busy
during batch 0's softmax and the DMA stream stays ahead of compute.

Sharding: data-parallel over B; each core handles B/8 = 2 batches.
"""

import numpy as np

import concourse.bacc as bacc
import concourse.tile as tile
import concourse.mybir as mybir
from concourse.bass_utils import run_bass_kernel_spmd
from concourse.masks import make_identity

B, T, Z, H, DK = 16, 4096, 1024, 16, 64
P = 128
NCORES = 8
BLOC = B // NCORES          # batches per core
ZC = Z // P                 # 8 z-chunks
TB = 512                    # t-block (one PSUM bank column span)
NTBG = 2                    # two groups of 4 t-blocks per batch
F32 = mybir.dt.float32
F16 = mybir.dt.float16
EXP = mybir.ActivationFunctionType.Exp
AX = mybir.AxisListType.X
MULT = mybir.AluOpType.mult


def build_nc():
    nc = bacc.Bacc(None, target_bir_lowering=False)

    o16 = nc.declare_dram_parameter("o16", [BLOC, T, Z], F16, isOutput=False)
    oT16 = nc.declare_dram_parameter("oT16", [2, P, ZC, TB], F16, isOutput=False)
    wv16 = nc.declare_dram_parameter("Wv16", [P, ZC, Z], F16, isOutput=False)
    wkq16 = nc.declare_dram_parameter("wkq16", [BLOC, P, ZC, H], F16, isOutput=False)
    bv128 = nc.declare_dram_parameter("bv128", [P, DK], F32, isOutput=False)
    dmask = nc.declare_dram_parameter("dmask", [P, 256], F16, isOutput=False)
    out = nc.declare_dram_parameter("out", [BLOC, Z], F32, isOutput=True)

    with tile.TileContext(nc) as tc:
        with (
            tc.tile_pool(name="const", bufs=1) as const,
            tc.tile_pool(name="abuf", bufs=2) as abuf,
            tc.tile_pool(name="atbuf", bufs=1) as atbuf,
            tc.tile_pool(name="stage", bufs=2) as stage,
            tc.tile_pool(name="small", bufs=2) as small,
            tc.tile_pool(name="scp", bufs=2, space="PSUM") as scp,
            tc.tile_pool(name="atp", bufs=3, space="PSUM") as atp,
            tc.tile_pool(name="xps", bufs=1, space="PSUM") as xps,
            tc.tile_pool(name="rp", bufs=1, space="PSUM") as rp,
        ):
            ident16 = const.tile([P, P], F16)
            make_identity(nc, ident16)
            identf = const.tile([P, P], F32)
            make_identity(nc, identf)
            onesf = const.tile([1, 1], F32)
            nc.vector.memset(onesf, 1.0)
            negones = const.tile([1, 1], F32)
            nc.vector.memset(negones, -1.0)

            # ------------- DMA schedule (single sync ring, FIFO starts) -----
            # tiny first, then b0 stream, then b1 stream with wv interleaved
            wkq_sb = []
            for b in range(BLOC):
                wkq_b = const.tile([P, ZC, H], F16, tag=f"wkq{b}")
                nc.sync.dma_start(out=wkq_b, in_=wkq16[b])
                wkq_sb.append(wkq_b)
            dmask_sb = const.tile([P, 256], F16)
            nc.sync.dma_start(out=dmask_sb, in_=dmask[:])
            bv_sb = const.tile([P, DK], F32)
            nc.sync.dma_start(out=bv_sb, in_=bv128[:])

            wv_sb = const.tile([P, ZC, Z], F16)
            a_sbs = []

            def load_blocks(b, blks, split=1):
                for blk in blks:
                    for s in range(split):
                        g0 = blk * 4 + s * 4 // split
                        g1 = blk * 4 + (s + 1) * 4 // split
                        nc.sync.dma_start(
                            out=a_sbs[b][:, g0:g1, :],
                            in_=o16[b, g0 * P : g1 * P, :].rearrange(
                                "(i zp) z -> zp i z", zp=P
                            ),
                        )

            for b in range(BLOC):
                a_sb = abuf.tile([P, 32, Z], F16, tag="a", name=f"a_sb{b}")
                a_sbs.append(a_sb)
            load_blocks(0, range(2), split=4)
            load_blocks(0, range(2, 8))
            load_blocks(1, range(0, 6))
            # host-pre-transposed A^T strips for b1/tbg1 j=2,3: replace 128 PE
            # transposes on the tail path; must land before S11's matmuls
            atx_sb = const.tile([P, 2, ZC, TB], F16)
            for jj in range(2):
                nc.sync.dma_start(out=atx_sb[:, jj], in_=oT16[jj])
            load_blocks(1, range(6, 8))
            # wv is consumed last (ctx matmuls run after both r passes)
            for zc in range(ZC):
                nc.sync.dma_start(out=wv_sb[:, zc, :], in_=wv16[:, zc, :])

            # ------------- stage emitters ----------------------------------
            def emit_tbg(b, tbg, mid=None, xbar_js=None):
                """A^T fills via PE transposes (j-outer), then col-tiled score
                matmuls; strips listed in xbar_js come pre-transposed in atx."""
                xbar_js = xbar_js or {}
                at_sb = atbuf.tile([P, 4, ZC, TB], F16, tag="at")
                sc_ps = scp.tile([P, TB], F32, tag="sc")
                nfill = 0
                for j in range(4):
                    if j in xbar_js:
                        continue
                    for zcp in range(4):
                        at_ps = atp.tile([P, 8, P], F16, tag="atp")
                        for zz in range(2):
                            zc = 2 * zcp + zz
                            for i in range(4):
                                gi = (tbg * 4 + j) * 4 + i
                                nc.tensor.transpose(
                                    at_ps[:, 4 * zz + i, :],
                                    a_sbs[b][:, gi, zc * P : (zc + 1) * P],
                                    ident16,
                                )
                        eng = nc.scalar if nfill % 3 == 2 else nc.vector
                        cp = eng.copy if nfill % 3 == 2 else eng.tensor_copy
                        cp(
                            out=at_sb[:, j, 2 * zcp : 2 * zcp + 2, :],
                            in_=at_ps.rearrange("p (zz i) c -> p zz (i c)", zz=2),
                        )
                        nfill += 1
                    if j == 0 and mid is not None:
                        mid()
                for zc in range(ZC):
                    for j in range(4):
                        rhs = (
                            atx_sb[:, xbar_js[j], zc, :]
                            if j in xbar_js
                            else at_sb[:, j, zc, :]
                        )
                        nc.tensor.matmul(
                            sc_ps[32 * j : 32 * j + 16, :],
                            wkq_sb[b][:, zc, :],
                            rhs,
                            start=(zc == 0),
                            stop=(zc == ZC - 1),
                            tile_position=(0, 32 * j),
                        )
                return sc_ps

            def emit_sm_maxes(b, sc_tiles):
                """per-partition chunk maxes (vector engine only)."""
                m_sb = small.tile([P, 2], F32, tag="m")
                for tbg in range(NTBG):
                    nc.vector.reduce_max(
                        m_sb[:, tbg : tbg + 1], sc_tiles[tbg], axis=AX
                    )
                mm1 = small.tile([P, 1], F32, tag="mm1")
                nc.vector.reduce_max(mm1, m_sb, axis=AX)
                return mm1

            def emit_sm_exp(b, sc_tiles, mm1):
                """per-head max combine + exp; scores are [32*(tb%4)+h, 512]."""
                p_sb = stage.tile([P, NTBG, TB], F16, tag="p")
                xs = xps.tile([P, P], F32, tag="xs")
                nc.tensor.transpose(xs[0:1, :], mm1, identf)
                M32 = small.tile([1, 32], F32, tag="M32")
                nc.vector.reduce_max(
                    M32, xs[0:1, :].rearrange("a (j c) -> a c j", j=4), axis=AX
                )
                Mr = small.tile([1, 4, 32], F32, tag="Mr")
                nc.vector.tensor_copy(Mr, M32.unsqueeze(1).to_broadcast((1, 4, 32)))
                xs = xps.tile([P, P], F32, tag="xs")
                nc.tensor.matmul(xs[:, 0:1], Mr, negones, start=True, stop=True)
                negM128 = small.tile([P, 1], F32, tag="negM128")
                nc.vector.tensor_copy(negM128, xs[:, 0:1])
                ls_sb = small.tile([P, 2], F32, tag="ls")
                for tbg in range(NTBG):
                    nc.scalar.activation(
                        out=p_sb[:, tbg, :],
                        in_=sc_tiles[tbg],
                        func=EXP,
                        bias=negM128,
                        scale=1.0,
                        accum_out=ls_sb[:, tbg : tbg + 1],
                    )
                return p_sb, ls_sb

            def emit_sm_l(b, ls_sb):
                """L = per-head sum of chunk sums; rinv128 = 1/L per partition."""
                ls1 = small.tile([P, 1], F32, tag="ls1")
                nc.vector.reduce_sum(ls1, ls_sb, axis=AX)
                xs = xps.tile([P, P], F32, tag="xs")
                nc.tensor.transpose(xs[0:1, :], ls1, identf)
                L32 = small.tile([1, 32], F32, tag="L32")
                nc.vector.reduce_sum(
                    L32, xs[0:1, :].rearrange("a (j c) -> a c j", j=4), axis=AX
                )
                rinv32 = small.tile([1, 32], F32, tag="rinv32")
                nc.vector.reciprocal(rinv32, L32)
                rinvr = small.tile([1, 4, 32], F32, tag="rinvr")
                nc.vector.tensor_copy(
                    rinvr, rinv32.unsqueeze(1).to_broadcast((1, 4, 32))
                )
                xs = xps.tile([P, P], F32, tag="xs")
                nc.tensor.matmul(xs[:, 0:1], rinvr, onesf, start=True, stop=True)
                rinv128 = small.tile([P, 1], F32, tag="rinv128")
                nc.vector.tensor_copy(rinv128, xs[:, 0:1])
                return rinv128

            def emit_pt(b, p_sb):
                """p natural (t on partitions) via PE transposes."""
                ptT = []
                for tbg in range(NTBG):
                    pt_ps = xps.tile([P, 4, P], F16, tag="ptT")
                    for i in range(4):
                        nc.tensor.transpose(
                            pt_ps[:, i, :],
                            p_sb[:, tbg, i * P : (i + 1) * P],
                            ident16,
                        )
                    pt_sb = stage.tile([P, 4, P], F16, tag=f"ptT{tbg}", bufs=1)
                    nc.vector.tensor_copy(pt_sb, pt_ps)
                    ptT.append(pt_sb)
                return ptT

            def emit_r(b, ptT):
                """r[h, z] col-tiled over z-quarters; rt = r^T chunks."""
                r_ps = rp.tile([P, 256], F32, tag="r")
                nmm = 0
                for tbg in range(NTBG):
                    for i in range(4):
                        for jt in range(4):
                            gi = (tbg * 4 + jt) * 4 + i
                            for j in range(4):
                                nc.tensor.matmul(
                                    r_ps[32 * j : 32 * j + 16, :],
                                    ptT[tbg][:, i, 32 * jt : 32 * jt + 16],
                                    a_sbs[b][:, gi, j * 256 : (j + 1) * 256],
                                    start=(nmm == 0),
                                    stop=(nmm == 31),
                                    tile_position=(0, 32 * j),
                                )
                            nmm += 1
                r16 = stage.tile([P, 256], F16, tag="r16", bufs=1)
                nc.vector.tensor_copy(r16, r_ps)
                rT_ps = xps.tile([P, 4, P], F16, tag="ptT")
                for half in range(2):
                    nc.tensor.transpose(
                        rT_ps[:, half, :],
                        r16[:, half * P : (half + 1) * P],
                        ident16,
                    )
                rt_sb = stage.tile([P, 2, P], F16, tag="rt")
                nc.vector.tensor_copy(rt_sb, rT_ps[:, 0:2, :])
                return rt_sb

            def emit_ctx(b, rt_sb, rinv128):
                cf_ps = rp.tile([P, 256], F32, tag="r", name="cf_ps")
                for zc in range(ZC):
                    half, zq = zc % 2, zc // 2
                    for j in range(4):
                        nc.tensor.matmul(
                            cf_ps[32 * j : 32 * j + 16, :],
                            rt_sb[:, half, 32 * zq : 32 * zq + 16],
                            wv_sb[:, zc, j * 256 : (j + 1) * 256],
                            start=(zc == 0),
                            stop=(zc == ZC - 1),
                            tile_position=(0, 32 * j),
                        )
                ctxm = stage.tile([P, 256], F32, tag="ctxm", bufs=1)
                nc.vector.scalar_tensor_tensor(
                    out=ctxm, in0=cf_ps, scalar=rinv128, in1=dmask_sb,
                    op0=MULT, op1=MULT,
                )
                ctxs = stage.tile([P, DK], F32, tag="ctxs")
                nc.vector.reduce_sum(
                    ctxs, ctxm.rearrange("p (g d) -> p d g", d=DK), axis=AX
                )
                nc.vector.tensor_add(out=ctxs, in0=ctxs, in1=bv_sb)
                outv = out[b].rearrange("(h d) -> h d", h=H)
                for j in range(4):
                    eng = nc.sync if j % 2 == 0 else nc.scalar
                    eng.dma_start(
                        out=outv[4 * j : 4 * j + 4, :],
                        in_=ctxs[36 * j : 36 * j + 4, :],
                    )

            # ------------- interleaved schedule ----------------------------
            # PE FIFO: T00 T01 [sm0 maxes] T10(sm0-exp after round 0) T11
            #          PT0 R0 [sm1 on V/S] CT0 PT1 R1 CT1
            sc00 = emit_tbg(0, 0)
            sc01 = emit_tbg(0, 1)
            sc0 = [sc00, sc01]
            mm1_0 = emit_sm_maxes(0, sc0)
            hold = {}

            def mid0():
                hold["p0"], hold["ls0"] = emit_sm_exp(0, sc0, mm1_0)
                hold["rinv0"] = emit_sm_l(0, hold["ls0"])

            sc10 = emit_tbg(1, 0, mid=mid0)
            sc11 = emit_tbg(1, 1, xbar_js={2: 0, 3: 1})
            sc1 = [sc10, sc11]
            mm1_1 = emit_sm_maxes(1, sc1)
            ptT0 = emit_pt(0, hold["p0"])
            p1, ls1 = emit_sm_exp(1, sc1, mm1_1)
            rt0 = emit_r(0, ptT0)
            ptT1 = emit_pt(1, p1)
            rt1 = emit_r(1, ptT1)
            rinv1 = emit_sm_l(1, ls1)
            emit_ctx(0, rt0, hold["rinv0"])
            emit_ctx(1, rt1, rinv1)

    nc.finalize()
    return nc


_NC_CACHE = {}


def _get_nc():
    if "nc" not in _NC_CACHE:
        _NC_CACHE["nc"] = build_nc()
    return _NC_CACHE["nc"]


def prep_inputs(o_all, o_last, Wk, Wv, Wq, bk, bv, bq):
    """Host-side shard + layout prep. Returns per-core input maps."""
    o_all = np.asarray(o_all, dtype=np.float32)
    o_last = np.asarray(o_last, dtype=np.float32)
    Wk = np.asarray(Wk, dtype=np.float32)
    Wv = np.asarray(Wv, dtype=np.float32)
    Wq = np.asarray(Wq, dtype=np.float32)
    bv = np.asarray(bv, dtype=np.float32)
    bq = np.asarray(bq, dtype=np.float32)

    # q for all batches, then wkq[z, h] = sum_d Wk[h,z,d] q[h,d]
    wq_flat = Wq.transpose(1, 0, 2).reshape(Z, Z)
    q_all = o_last[:, 0, :] @ wq_flat + bq.reshape(Z)          # [B, Z]
    wkq_all = np.einsum(
        "hzd,bhd->bzh", Wk, q_all.reshape(B, H, DK), optimize=True
    )                                                           # [B, Z, H]

    wv_flat = Wv.transpose(1, 0, 2).reshape(Z, Z)
    wv16 = np.ascontiguousarray(
        wv_flat.reshape(ZC, P, Z).transpose(1, 0, 2)
    ).astype(np.float16)

    bv128 = np.zeros((P, DK), dtype=np.float32)
    dmask = np.zeros((P, 256), dtype=np.float16)
    for h in range(H):
        j, r = h // 4, h % 4
        bv128[36 * j + r] = bv[h]
        dmask[32 * j + h, DK * r : DK * (r + 1)] = 1.0

    in_maps = []
    for c in range(NCORES):
        sl = slice(c * BLOC, (c + 1) * BLOC)
        wkq16 = np.ascontiguousarray(
            wkq_all[sl].reshape(BLOC, ZC, P, H).transpose(0, 2, 1, 3)
        ).astype(np.float16)
        o16c = o_all[sl].astype(np.float16)
        # pre-transposed A^T strips: batch 1, t-blocks 6 and 7
        oT = np.ascontiguousarray(
            np.stack([o16c[1, tb * TB : (tb + 1) * TB, :] for tb in (6, 7)])
            .reshape(2, TB, ZC, P)
            .transpose(0, 3, 2, 1)                    # [2, P, ZC, TB]
        )
        in_maps.append(
            {
                "o16": o16c,
                "oT16": oT,
                "Wv16": wv16,
                "wkq16": wkq16,
                "bv128": bv128,
                "dmask": dmask,
            }
        )
    return in_maps


def kernel(o_all, o_last, Wk, Wv, Wq, bk, bv, bq, _trace=False, _trace_kwargs=None):
    nc = _get_nc()
    in_maps = prep_inputs(o_all, o_last, Wk, Wv, Wq, bk, bv, bq)
    res = run_bass_kernel_spmd(
        nc, in_maps, core_ids=list(range(NCORES)), trace=_trace,
        **(_trace_kwargs or {}),
    )
    outs = [r["out"] for r in res.results]
    full = np.concatenate(outs, axis=0).reshape(B, 1, Z)
    if _trace:
        kernel.last_result = res
    return full


# revision 49
# speedup vs baseline: 1.0371x; 1.0030x over previous
"""MultiHeadTimeDimensionAttention kernel for Trainium2 (8 NeuronCores).

Math (per batch b):
  q[h,d]      = o_last[b] . Wq[h,:,d] + bq[h,d]          (host, 0.4% of FLOPs)
  wkq[z,h]    = sum_d Wk[h,z,d] q[h,d]                   (host)
  scores[t,h] = sum_z o_all[b,t,z] * wkq[z,h]            (device, bk drops: softmax-invariant)
  p = exp(scores - max_t), L = sum_t p
  r[h,z]      = sum_t p[t,h] * o_all[b,t,z]
  ctx[h,d]    = (sum_z r[h,z] Wv[h,z,d]) / L[h] + bv[h,d]

Device layout: scores^T kept in a (tb,h)-packed [128, 512] PSUM layout via
column-tiled (tile_position) M=16 matmuls, 4 t-blocks concurrent on the PE
array.  Softmax runs at full 128-partition parallelism; cross-partition
head reductions go through tiny PE transposes + a K=1 broadcast matmul.
A^T tiles are produced on-PE from the (single) natural-layout copy of
o_all; fp16 everywhere on the PE, fp32 PSUM/softmax.

The two batches per core are software-pipelined: batch 1's transpose fills
and score matmuls are emitted between batch 0's stages so the PE stays busy
during batch 0's softmax and the DMA stream stays ahead of compute.

Sharding: data-parallel over B; each core handles B/8 = 2 batches.
"""

import numpy as np

import concourse.bacc as bacc
import concourse.tile as tile
import concourse.mybir as mybir
from concourse.bass_utils import run_bass_kernel_spmd
from concourse.masks import make_identity

B, T, Z, H, DK = 16, 4096, 1024, 16, 64
P = 128
NCORES = 8
BLOC = B // NCORES          # batches per core
ZC = Z // P                 # 8 z-chunks
TB = 512                    # t-block (one PSUM bank column span)
NTBG = 2                    # two groups of 4 t-blocks per batch
F32 = mybir.dt.float32
F16 = mybir.dt.float16
EXP = mybir.ActivationFunctionType.Exp
AX = mybir.AxisListType.X
MULT = mybir.AluOpType.mult


def build_nc():
    nc = bacc.Bacc(None, target_bir_lowering=False)

    o16 = nc.declare_dram_parameter("o16", [BLOC, T, Z], F16, isOutput=False)
    oT16 = nc.declare_dram_parameter("oT16", [2, P, ZC, TB], F16, isOutput=False)
    wv16 = nc.declare_dram_parameter("Wv16", [P, ZC, Z], F16, isOutput=False)
    wkq16 = nc.declare_dram_parameter("wkq16", [BLOC, P, ZC, H], F16, isOutput=False)
    bv128 = nc.declare_dram_parameter("bv128", [P, DK], F32, isOutput=False)
    dmask = nc.declare_dram_parameter("dmask", [P, 256], F16, isOutput=False)
    out = nc.declare_dram_parameter("out", [BLOC, Z], F32, isOutput=True)

    with tile.TileContext(nc) as tc:
        with (
            tc.tile_pool(name="const", bufs=1) as const,
            tc.tile_pool(name="abuf", bufs=2) as abuf,
            tc.tile_pool(name="atbuf", bufs=1) as atbuf,
            tc.tile_pool(name="stage", bufs=2) as stage,
            tc.tile_pool(name="small", bufs=2) as small,
            tc.tile_pool(name="scp", bufs=2, space="PSUM") as scp,
            tc.tile_pool(name="atp", bufs=3, space="PSUM") as atp,
            tc.tile_pool(name="xps", bufs=1, space="PSUM") as xps,
            tc.tile_pool(name="rp", bufs=1, space="PSUM") as rp,
        ):
            ident16 = const.tile([P, P], F16)
            make_identity(nc, ident16)
            identf = const.tile([P, P], F32)
            make_identity(nc, identf)
            onesf = const.tile([1, 1], F32)
            nc.vector.memset(onesf, 1.0)
            negones = const.tile([1, 1], F32)
            nc.vector.memset(negones, -1.0)

            # ------------- DMA schedule (single sync ring, FIFO starts) -----
            # tiny first, then b0 stream, then b1 stream with wv interleaved
            wkq_sb = []
            for b in range(BLOC):
                wkq_b = const.tile([P, ZC, H], F16, tag=f"wkq{b}")
                nc.sync.dma_start(out=wkq_b, in_=wkq16[b])
                wkq_sb.append(wkq_b)
            dmask_sb = const.tile([P, 256], F16)
            nc.sync.dma_start(out=dmask_sb, in_=dmask[:])
            bv_sb = const.tile([P, DK], F32)
            nc.sync.dma_start(out=bv_sb, in_=bv128[:])

            wv_sb = const.tile([P, ZC, Z], F16)
            a_sbs = []

            def load_blocks(b, blks, split=1):
                for blk in blks:
                    for s in range(split):
                        g0 = blk * 4 + s * 4 // split
                        g1 = blk * 4 + (s + 1) * 4 // split
                        nc.sync.dma_start(
                            out=a_sbs[b][:, g0:g1, :],
                            in_=o16[b, g0 * P : g1 * P, :].rearrange(
                                "(i zp) z -> zp i z", zp=P
                            ),
                        )

            for b in range(BLOC):
                a_sb = abuf.tile([P, 32, Z], F16, tag="a", name=f"a_sb{b}")
                a_sbs.append(a_sb)
            load_blocks(0, range(2), split=4)
            load_blocks(0, range(2, 8))
            load_blocks(1, range(0, 6))
            # host-pre-transposed A^T strips for b1/tbg1 j=2,3: replace 128 PE
            # transposes on the tail path; must land before S11's matmuls
            atx_sb = const.tile([P, 2, ZC, TB], F16)
            for jj in range(2):
                nc.sync.dma_start(out=atx_sb[:, jj], in_=oT16[jj])
            for zc in range(ZC):
                nc.sync.dma_start(out=wv_sb[:, zc, :], in_=wv16[:, zc, :])
            load_blocks(1, range(6, 8))

            # ------------- stage emitters ----------------------------------
            def emit_tbg(b, tbg, mid=None, xbar_js=None):
                """A^T fills via PE transposes (j-outer), then col-tiled score
                matmuls; strips listed in xbar_js come pre-transposed in atx."""
                xbar_js = xbar_js or {}
                at_sb = atbuf.tile([P, 4, ZC, TB], F16, tag="at")
                sc_ps = scp.tile([P, TB], F32, tag="sc")
                nfill = 0
                for j in range(4):
                    if j in xbar_js:
                        continue
                    for zcp in range(4):
                        at_ps = atp.tile([P, 8, P], F16, tag="atp")
                        for zz in range(2):
                            zc = 2 * zcp + zz
                            for i in range(4):
                                gi = (tbg * 4 + j) * 4 + i
                                nc.tensor.transpose(
                                    at_ps[:, 4 * zz + i, :],
                                    a_sbs[b][:, gi, zc * P : (zc + 1) * P],
                                    ident16,
                                )
                        eng = nc.scalar if nfill % 3 == 2 else nc.vector
                        cp = eng.copy if nfill % 3 == 2 else eng.tensor_copy
                        cp(
                            out=at_sb[:, j, 2 * zcp : 2 * zcp + 2, :],
                            in_=at_ps.rearrange("p (zz i) c -> p zz (i c)", zz=2),
                        )
                        nfill += 1
                    if j == 0 and mid is not None:
                        mid()
                for zc in range(ZC):
                    for j in range(4):
                        rhs = (
                            atx_sb[:, xbar_js[j], zc, :]
                            if j in xbar_js
                            else at_sb[:, j, zc, :]
                        )
                        nc.tensor.matmul(
                            sc_ps[32 * j : 32 * j + 16, :],
                            wkq_sb[b][:, zc, :],
                            rhs,
                            start=(zc == 0),
                            stop=(zc == ZC - 1),
                            tile_position=(0, 32 * j),
                        )
                return sc_ps

            def emit_sm_maxes(b, sc_tiles):
                """per-partition chunk maxes (vector engine only)."""
                m_sb = small.tile([P, 2], F32, tag="m")
                for tbg in range(NTBG):
                    nc.vector.reduce_max(
                        m_sb[:, tbg : tbg + 1], sc_tiles[tbg], axis=AX
                    )
                mm1 = small.tile([P, 1], F32, tag="mm1")
                nc.vector.reduce_max(mm1, m_sb, axis=AX)
                return mm1

            def emit_sm_exp(b, sc_tiles, mm1):
                """per-head max combine + exp; scores are [32*(tb%4)+h, 512]."""
                p_sb = stage.tile([P, NTBG, TB], F16, tag="p")
                xs = xps.tile([P, P], F32, tag="xs")
                nc.tensor.transpose(xs[0:1, :], mm1, identf)
                M32 = small.tile([1, 32], F32, tag="M32")
                nc.vector.reduce_max(
                    M32, xs[0:1, :].rearrange("a (j c) -> a c j", j=4), axis=AX
                )
                Mr = small.tile([1, 4, 32], F32, tag="Mr")
                nc.vector.tensor_copy(Mr, M32.unsqueeze(1).to_broadcast((1, 4, 32)))
                xs = xps.tile([P, P], F32, tag="xs")
                nc.tensor.matmul(xs[:, 0:1], Mr, negones, start=True, stop=True)
                negM128 = small.tile([P, 1], F32, tag="negM128")
                nc.vector.tensor_copy(negM128, xs[:, 0:1])
                ls_sb = small.tile([P, 2], F32, tag="ls")
                for tbg in range(NTBG):
                    nc.scalar.activation(
                        out=p_sb[:, tbg, :],
                        in_=sc_tiles[tbg],
                        func=EXP,
                        bias=negM128,
                        scale=1.0,
                        accum_out=ls_sb[:, tbg : tbg + 1],
                    )
                return p_sb, ls_sb

            def emit_sm_l(b, ls_sb):
                """L = per-head sum of chunk sums; rinv128 = 1/L per partition."""
                ls1 = small.tile([P, 1], F32, tag="ls1")
                nc.vector.reduce_sum(ls1, ls_sb, axis=AX)
                xs = xps.tile([P, P], F32, tag="xs")
                nc.tensor.transpose(xs[0:1, :], ls1, identf)
                L32 = small.tile([1, 32], F32, tag="L32")
                nc.vector.reduce_sum(
                    L32, xs[0:1, :].rearrange("a (j c) -> a c j", j=4), axis=AX
                )
                rinv32 = small.tile([1, 32], F32, tag="rinv32")
                nc.vector.reciprocal(rinv32, L32)
                rinvr = small.tile([1, 4, 32], F32, tag="rinvr")
                nc.vector.tensor_copy(
                    rinvr, rinv32.unsqueeze(1).to_broadcast((1, 4, 32))
                )
                xs = xps.tile([P, P], F32, tag="xs")
                nc.tensor.matmul(xs[:, 0:1], rinvr, onesf, start=True, stop=True)
                rinv128 = small.tile([P, 1], F32, tag="rinv128")
                nc.vector.tensor_copy(rinv128, xs[:, 0:1])
                return rinv128

            def emit_pt(b, p_sb):
                """p natural (t on partitions) via PE transposes."""
                ptT = []
                for tbg in range(NTBG):
                    pt_ps = xps.tile([P, 4, P], F16, tag="ptT")
                    for i in range(4):
                        nc.tensor.transpose(
                            pt_ps[:, i, :],
                            p_sb[:, tbg, i * P : (i + 1) * P],
                            ident16,
                        )
                    pt_sb = stage.tile([P, 4, P], F16, tag=f"ptT{tbg}", bufs=1)
                    nc.vector.tensor_copy(pt_sb, pt_ps)
                    ptT.append(pt_sb)
                return ptT

            def emit_r(b, ptT):
                """r[h, z] col-tiled over z-quarters; rt = r^T chunks."""
                r_ps = rp.tile([P, 256], F32, tag="r")
                nmm = 0
                for tbg in range(NTBG):
                    for i in range(4):
                        for jt in range(4):
                            gi = (tbg * 4 + jt) * 4 + i
                            for j in range(4):
                                nc.tensor.matmul(
                                    r_ps[32 * j : 32 * j + 16, :],
                                    ptT[tbg][:, i, 32 * jt : 32 * jt + 16],
                                    a_sbs[b][:, gi, j * 256 : (j + 1) * 256],
                                    start=(nmm == 0),
                                    stop=(nmm == 31),
                                    tile_position=(0, 32 * j),
                                )
                            nmm += 1
                r16 = stage.tile([P, 256], F16, tag="r16", bufs=1)
                nc.vector.tensor_copy(r16, r_ps)
                rT_ps = xps.tile([P, 4, P], F16, tag="ptT")
                for half in range(2):
                    nc.tensor.transpose(
                        rT_ps[:, half, :],
                        r16[:, half * P : (half + 1) * P],
                        ident16,
                    )
                rt_sb = stage.tile([P, 2, P], F16, tag="rt")
                nc.vector.tensor_copy(rt_sb, rT_ps[:, 0:2, :])
                return rt_sb

            def emit_ctx(b, rt_sb, rinv128):
                cf_full = scp.tile([P, TB], F32, tag="sc", name="cf_ps")
                cf_ps = cf_full[:, 0:256]
                for zc in range(ZC):
                    half, zq = zc % 2, zc // 2
                    for j in range(4):
                        nc.tensor.matmul(
                            cf_ps[32 * j : 32 * j + 16, :],
                            rt_sb[:, half, 32 * zq : 32 * zq + 16],
                            wv_sb[:, zc, j * 256 : (j + 1) * 256],
                            start=(zc == 0),
                            stop=(zc == ZC - 1),
                            tile_position=(0, 32 * j),
                        )
                ctxm = stage.tile([P, 256], F32, tag="ctxm", bufs=1)
                nc.vector.scalar_tensor_tensor(
                    out=ctxm, in0=cf_ps, scalar=rinv128, in1=dmask_sb,
                    op0=MULT, op1=MULT,
                )
                ctxs = stage.tile([P, DK], F32, tag="ctxs")
                nc.vector.reduce_sum(
                    ctxs, ctxm.rearrange("p (g d) -> p d g", d=DK), axis=AX
                )
                nc.vector.tensor_add(out=ctxs, in0=ctxs, in1=bv_sb)
                outv = out[b].rearrange("(h d) -> h d", h=H)
                for j in range(4):
                    eng = nc.sync if j % 2 == 0 else nc.scalar
                    eng.dma_start(
                        out=outv[4 * j : 4 * j + 4, :],
                        in_=ctxs[36 * j : 36 * j + 4, :],
                    )

            # ------------- interleaved schedule ----------------------------
            # PE FIFO: T00 T01 [sm0 maxes] T10(sm0-exp after round 0) T11
            #          PT0 R0 [sm1 on V/S] CT0 PT1 R1 CT1
            sc00 = emit_tbg(0, 0)
            sc01 = emit_tbg(0, 1)
            sc0 = [sc00, sc01]
            mm1_0 = emit_sm_maxes(0, sc0)
            hold = {}

            def mid0():
                hold["p0"], hold["ls0"] = emit_sm_exp(0, sc0, mm1_0)
                hold["rinv0"] = emit_sm_l(0, hold["ls0"])

            sc10 = emit_tbg(1, 0, mid=mid0)
            sc11 = emit_tbg(1, 1, xbar_js={2: 0, 3: 1})
            sc1 = [sc10, sc11]
            mm1_1 = emit_sm_maxes(1, sc1)
            ptT0 = emit_pt(0, hold["p0"])
            p1, ls1 = emit_sm_exp(1, sc1, mm1_1)
            rt0 = emit_r(0, ptT0)
            emit_ctx(0, rt0, hold["rinv0"])
            ptT1 = emit_pt(1, p1)
            rt1 = emit_r(1, ptT1)
            rinv1 = emit_sm_l(1, ls1)
            emit_ctx(1, rt1, rinv1)

    nc.finalize()
    return nc


_NC_CACHE = {}


def _get_nc():
    if "nc" not in _NC_CACHE:
        _NC_CACHE["nc"] = build_nc()
    return _NC_CACHE["nc"]


def prep_inputs(o_all, o_last, Wk, Wv, Wq, bk, bv, bq):
    """Host-side shard + layout prep. Returns per-core input maps."""
    o_all = np.asarray(o_all, dtype=np.float32)
    o_last = np.asarray(o_last, dtype=np.float32)
    Wk = np.asarray(Wk, dtype=np.float32)
    Wv = np.asarray(Wv, dtype=np.float32)
    Wq = np.asarray(Wq, dtype=np.float32)
    bv = np.asarray(bv, dtype=np.float32)
    bq = np.asarray(bq, dtype=np.float32)

    # q for all batches, then wkq[z, h] = sum_d Wk[h,z,d] q[h,d]
    wq_flat = Wq.transpose(1, 0, 2).reshape(Z, Z)
    q_all = o_last[:, 0, :] @ wq_flat + bq.reshape(Z)          # [B, Z]
    wkq_all = np.einsum(
        "hzd,bhd->bzh", Wk, q_all.reshape(B, H, DK), optimize=True
    )                                                           # [B, Z, H]

    wv_flat = Wv.transpose(1, 0, 2).reshape(Z, Z)
    wv16 = np.ascontiguousarray(
        wv_flat.reshape(ZC, P, Z).transpose(1, 0, 2)
    ).astype(np.float16)

    bv128 = np.zeros((P, DK), dtype=np.float32)
    dmask = np.zeros((P, 256), dtype=np.float16)
    for h in range(H):
        j, r = h // 4, h % 4
        bv128[36 * j + r] = bv[h]
        dmask[32 * j + h, DK * r : DK * (r + 1)] = 1.0

    in_maps = []
    for c in range(NCORES):
        sl = slice(c * BLOC, (c + 1) * BLOC)
        wkq16 = np.ascontiguousarray(
            wkq_all[sl].reshape(BLOC, ZC, P, H).transpose(0, 2, 1, 3)
        ).astype(np.float16)
        o16c = o_all[sl].astype(np.float16)
        # pre-transposed A^T strips: batch 1, t-blocks 6 and 7
        oT = np.ascontiguousarray(
            np.stack([o16c[1, tb * TB : (tb + 1) * TB, :] for tb in (6, 7)])
            .reshape(2, TB, ZC, P)
            .transpose(0, 3, 2, 1)                    # [2, P, ZC, TB]
        )
        in_maps.append(
            {
                "o16": o16c,
                "oT16": oT,
                "Wv16": wv16,
                "wkq16": wkq16,
                "bv128": bv128,
                "dmask": dmask,
            }
        )
    return in_maps


def kernel(o_all, o_last, Wk, Wv, Wq, bk, bv, bq, _trace=False, _trace_kwargs=None):
    nc = _get_nc()
    in_maps = prep_inputs(o_all, o_last, Wk, Wv, Wq, bk, bv, bq)
    res = run_bass_kernel_spmd(
        nc, in_maps, core_ids=list(range(NCORES)), trace=_trace,
        **(_trace_kwargs or {}),
    )
    outs = [r["out"] for r in res.results]
    full = np.concatenate(outs, axis=0).reshape(B, 1, Z)
    if _trace:
        kernel.last_result = res
    return full
